# revision 18
# baseline (speedup 1.0000x reference)
"""Log2Quantizer Trainium2 kernel (raw Bass, no Tile).

Math: the reference's sort/std/rank machinery is dead code (bit_token is
unconditionally overwritten with n_bits), so the computation reduces to:
    delta[b,t] = max over (h,c) of x[b,h,t,c]
    out = delta * 2^(round(log2(max(x/delta, 1e-8))))
i.e. snap x/delta to the nearest power of two in log space, rescale by delta.

Division-route bit-trick (no transcendentals), exact on the fp32-internal DVE:
    q  = x * (1 / (delta*sqrt2))             (reciprocal is IEEE 1/x on trn2)
    p2 = bitcast_f32(bits(q) & 0x7F800000)   # 2^floor(log2 q) = 2^(k-1)
    out = p2 * (2*delta)                     # fp32 mult by 2^k, exact
round(log2(x/delta)) = floor(log2(x/(delta*sqrt2))) + 1, so flooring q to its
exponent implements the rounding; x==0 gives q=0 -> p2=+0.0 -> out=0 (the
reference's 1e-8 ratio clamp yields delta*2^-27 ~ 7e-9 there; abs err 7e-9).

Sharding: data-parallel over batch dim b (8 rows -> 8 cores), no comms.
Layout: t split into chunks; partition dim = t-block so each partition line is
one contiguous run per h in DRAM. Per-token scalars broadcast along the free
(h, c) dims with stride-0 APs.

Engine split (DVE was the bottleneck at 113us busy all-DVE; gp free-dim
reduce and TensorScalarPtr are unsupported, so gp only takes the final
tensor_tensor mult):
  DVE:    R1+R2 reduces, recip smalls, M1 mult, AND mask (~9us/chunk)
  GpSimd: M2 final fp32 mult (tensor_tensor)             (~5us/chunk)
  Sync:   HWDGE DMAs, double buffered
Per-chunk chain: load -> R1,R2,smalls,M1,AND (dve) -> M2 (gp) -> store
Sync protocol (every instruction carries at most one sem update):
  dve_sem: +1 by each DVE op (7/chunk); self-fences + gp's waits
  v_sem:   +1 by M2 (gp); gates stores, loads, and DVE's small-tile reuse
  load_sem/store_sem: parity-split per-DMA 16-increments
"""

from contextlib import ExitStack

import numpy as np

import concourse.bass as bass
import concourse.mybir as mybir
from concourse.bass_utils import run_bass_kernel_spmd

B, H, T, C = 8, 12, 4096, 64
N_CORES = 8
P = 128          # SBUF partitions
TC = 512         # tokens per chunk (pipeline granularity)

SQRT2 = 1.4142135623730951
EXP_MASK = 0x7F800000

DVE_INCS = 7     # dve_sem increments per chunk (R1, R2, tf, inv, d2, M1, AND)

_nc_cache = {}


def _build_nc():
    if "nc" in _nc_cache:
        return _nc_cache["nc"]
    f32 = mybir.dt.float32
    i32 = mybir.dt.int32
    OP = mybir.AluOpType
    X = mybir.AxisListType.X

    nc = bass.Bass()
    x_in = nc.declare_dram_parameter("x", [H, T, C], f32, isOutput=False)
    y_out = nc.declare_dram_parameter("y", [H, T, C], f32, isOutput=True)

    n_chunks = T // TC
    tt = TC // P
    FREE = H * tt * C

    def src_ap(ci):
        return x_in[:, ci * TC : (ci + 1) * TC, :].rearrange(
            "h (p q) c -> p h (q c)", p=P
        )

    def dst_ap(ci):
        return y_out[:, ci * TC : (ci + 1) * TC, :].rearrange(
            "h (p q) c -> p h (q c)", p=P
        )

    with ExitStack() as ctx:
        xt = [ctx.enter_context(nc.sbuf_tensor(f"xt{j}", [P, FREE], f32)) for j in (0, 1)]
        wt = [ctx.enter_context(nc.sbuf_tensor(f"wt{j}", [P, FREE], f32)) for j in (0, 1)]
        r1 = ctx.enter_context(nc.sbuf_tensor("r1", [P, H * tt], f32))
        delta = ctx.enter_context(nc.sbuf_tensor("delta", [P, tt], f32))
        tf = ctx.enter_context(nc.sbuf_tensor("tf", [P, tt], f32))
        inv = ctx.enter_context(nc.sbuf_tensor("inv", [P, tt], f32))
        d2 = ctx.enter_context(nc.sbuf_tensor("d2", [P, tt], f32))

        load_sem = [ctx.enter_context(nc.semaphore(f"load_sem{j}")) for j in (0, 1)]
        store_sem = [ctx.enter_context(nc.semaphore(f"store_sem{j}")) for j in (0, 1)]
        v_sem = ctx.enter_context(nc.semaphore("v_sem"))
        dve_sem = ctx.enter_context(nc.semaphore("dve_sem"))

        block = ctx.enter_context(nc.Block())

        @block.sync
        def _(sync):
            for ci in range(n_chunks):
                sync.dma_start(out=xt[ci % 2][:], in_=src_ap(ci)).then_inc(
                    load_sem[ci % 2], 16
                )
                if ci >= 1:
                    sync.wait_ge(v_sem, ci)
                    sync.dma_start(
                        out=dst_ap(ci - 1), in_=wt[(ci - 1) % 2][:]
                    ).then_inc(store_sem[(ci - 1) % 2], 16)
            sync.wait_ge(v_sem, n_chunks)
            sync.dma_start(
                out=dst_ap(n_chunks - 1), in_=wt[(n_chunks - 1) % 2][:]
            ).then_inc(store_sem[(n_chunks - 1) % 2], 16)

        @block.gpsimd
        def _(gp):
            for ci in range(n_chunks):
                j = ci % 2
                xt4 = xt[j][:].rearrange("p (h q c) -> p h q c", h=H, c=C)
                wt4 = wt[j][:].rearrange("p (h q c) -> p h q c", h=H, c=C)

                # M2: out = p2 * 2*delta  (xt -> wt; wt dead after AND)
                d2_b = d2[:].unsqueeze(1).unsqueeze(3).broadcast_to([P, H, tt, C])
                gp.wait_ge(dve_sem, DVE_INCS * (ci + 1))        # AND(ci) done
                gp.tensor_tensor(out=wt4, in0=xt4, in1=d2_b, op=OP.mult).then_inc(
                    v_sem, 1
                )

        @block.vector
        def _(vector):
            for ci in range(n_chunks):
                j = ci % 2
                xt4 = xt[j][:].rearrange("p (h q c) -> p h q c", h=H, c=C)
                wt4 = wt[j][:].rearrange("p (h q c) -> p h q c", h=H, c=C)
                r13 = r1[:].rearrange("p (h q) -> p h q", h=H)

                if ci >= 1:
                    vector.wait_ge(v_sem, ci)      # M2(ci-1) done (small-tile WAR)
                    vector.wait_ge(dve_sem, DVE_INCS * ci)      # self-fence
                if ci >= 2:
                    vector.wait_ge(store_sem[j], 16 * (ci // 2))  # wt free
                vector.wait_ge(load_sem[j], 16 * (ci // 2 + 1))   # xt loaded

                # R1: per-(token,h) max over c; R2: delta = max over h
                vector.reduce_max(out=r13, in_=xt4, axis=X).then_inc(dve_sem, 1)
                vector.wait_ge(dve_sem, DVE_INCS * ci + 1)
                vector.reduce_max(
                    out=delta[:], in_=r13.transpose([0, 2, 1]), axis=X
                ).then_inc(dve_sem, 1)
                # per-token scalars
                vector.wait_ge(dve_sem, DVE_INCS * ci + 2)
                vector.tensor_scalar_mul(tf[:], delta[:], SQRT2).then_inc(dve_sem, 1)
                vector.wait_ge(dve_sem, DVE_INCS * ci + 3)
                vector.reciprocal(inv[:], tf[:]).then_inc(dve_sem, 1)
                vector.tensor_scalar_mul(d2[:], delta[:], 2.0).then_inc(dve_sem, 1)

                inv_b = inv[:].unsqueeze(1).unsqueeze(3).broadcast_to([P, H, tt, C])

                # M1: q = x * inv
                vector.wait_ge(dve_sem, DVE_INCS * ci + 5)
                vector.tensor_tensor(out=wt4, in0=xt4, in1=inv_b, op=OP.mult).then_inc(
                    dve_sem, 1
                )
                # AND: p2 = bits(q) & 0x7F800000  (wt -> xt, xt dead after M1)
                vector.wait_ge(dve_sem, DVE_INCS * ci + 6)
                vector.tensor_scalar(
                    out=xt[j][:].bitcast(i32),
                    in0=wt[j][:].bitcast(i32),
                    scalar1=EXP_MASK,
                    scalar2=None,
                    op0=OP.bitwise_and,
                ).then_inc(dve_sem, 1)

    _nc_cache["nc"] = nc
    return nc


def kernel(x: np.ndarray) -> np.ndarray:
    assert x.shape == (B, H, T, C) and x.dtype == np.float32
    nc = _build_nc()
    in_maps = [{"x": np.ascontiguousarray(x[i])} for i in range(N_CORES)]
    res = run_bass_kernel_spmd(nc, in_maps, list(range(N_CORES)))
    out = np.stack([res.results[i]["y"] for i in range(N_CORES)], axis=0)
    return out


# revision 22
# speedup vs baseline: 1.1470x; 1.1470x over previous
"""Log2Quantizer Trainium2 kernel (raw Bass, no Tile).

Math: the reference's sort/std/rank machinery is dead code (bit_token is
unconditionally overwritten with n_bits), so the computation reduces to:
    delta[b,t] = max over (h,c) of x[b,h,t,c]
    out = delta * 2^(round(log2(max(x/delta, 1e-8))))
i.e. snap x/delta to the nearest power of two in log space, rescale by delta.

Division-route bit-trick (no transcendentals), exact on the fp32-internal DVE:
    q  = x * (1 / (delta*sqrt2))             (reciprocal is IEEE 1/x on trn2)
    p2 = bitcast_f32(bits(q) & 0x7F800000)   # 2^floor(log2 q) = 2^(k-1)
    out = p2 * (2*delta)                     # fp32 mult by 2^k, exact
round(log2(x/delta)) = floor(log2(x/(delta*sqrt2))) + 1, so flooring q to its
exponent implements the rounding; x==0 gives q=0 -> p2=+0.0 -> out=0 (the
reference's 1e-8 ratio clamp yields delta*2^-27 ~ 7e-9 there; abs err 7e-9).

Sharding: data-parallel over batch dim b (8 rows -> 8 cores), no comms.
Layout: t split into chunks; partition dim = t-block so each partition line is
one contiguous run per h in DRAM. Per-token scalars broadcast along the free
(h, c) dims with stride-0 APs.

Engine split (DVE was the bottleneck at 113us busy all-DVE; gp free-dim
reduce and TensorScalarPtr are unsupported, so gp only takes the final
tensor_tensor mult):
  DVE:    R1+R2 reduces, recip smalls, M1 mult, AND mask (~9us/chunk)
  GpSimd: M2 final fp32 mult (tensor_tensor)             (~5us/chunk)
  Sync:   HWDGE DMAs, double buffered
Per-chunk chain: load -> R1,R2,smalls,M1,AND (dve) -> M2 (gp) -> store
Sync protocol (every instruction carries at most one sem update):
  dve_sem: +1 by each DVE op (7/chunk); self-fences + gp's waits
  v_sem:   +1 by M2 (gp); gates stores, loads, and DVE's small-tile reuse
  load_sem/store_sem: parity-split per-DMA 16-increments
"""

from contextlib import ExitStack

import numpy as np

import concourse.bass as bass
import concourse.mybir as mybir
from concourse.bass_utils import run_bass_kernel_spmd

B, H, T, C = 8, 12, 4096, 64
N_CORES = 8
P = 128          # SBUF partitions
TC = 512         # tokens per chunk (pipeline granularity)

SQRT2 = 1.4142135623730951
EXP_MASK = 0x7F800000

DVE_INCS = 7     # dve_sem increments per chunk (R1, R2, tf, inv, d2, M1, AND)

_nc_cache = {}


def _build_nc():
    if "nc" in _nc_cache:
        return _nc_cache["nc"]
    f32 = mybir.dt.float32
    i32 = mybir.dt.int32
    OP = mybir.AluOpType
    X = mybir.AxisListType.X

    nc = bass.Bass()
    x_in = nc.declare_dram_parameter("x", [H, T, C], f32, isOutput=False)
    y_out = nc.declare_dram_parameter("y", [H, T, C], f32, isOutput=True)

    n_chunks = T // TC
    tt = TC // P
    FREE = H * tt * C

    def src_ap(ci):
        return x_in[:, ci * TC : (ci + 1) * TC, :].rearrange(
            "h (p q) c -> p h (q c)", p=P
        )

    def dst_ap(ci):
        return y_out[:, ci * TC : (ci + 1) * TC, :].rearrange(
            "h (p q) c -> p h (q c)", p=P
        )

    with ExitStack() as ctx:
        xt = [ctx.enter_context(nc.sbuf_tensor(f"xt{j}", [P, FREE], f32)) for j in (0, 1)]
        wt = [ctx.enter_context(nc.sbuf_tensor(f"wt{j}", [P, FREE], f32)) for j in (0, 1)]
        r1 = ctx.enter_context(nc.sbuf_tensor("r1", [P, H * tt], f32))
        delta = ctx.enter_context(nc.sbuf_tensor("delta", [P, tt], f32))
        tf = ctx.enter_context(nc.sbuf_tensor("tf", [P, tt], f32))
        inv = ctx.enter_context(nc.sbuf_tensor("inv", [P, tt], f32))
        # d2 is read cross-engine by gp's M2 -> parity-split so the next DVE
        # chunk never has to wait for M2
        d2 = [ctx.enter_context(nc.sbuf_tensor(f"d2_{j}", [P, tt], f32)) for j in (0, 1)]

        load_sem = [ctx.enter_context(nc.semaphore(f"load_sem{j}")) for j in (0, 1)]
        store_sem = [ctx.enter_context(nc.semaphore(f"store_sem{j}")) for j in (0, 1)]
        v_sem = ctx.enter_context(nc.semaphore("v_sem"))
        dve_sem = ctx.enter_context(nc.semaphore("dve_sem"))

        block = ctx.enter_context(nc.Block())

        @block.sync
        def _(sync):
            for ci in range(n_chunks):
                sync.dma_start(out=xt[ci % 2][:], in_=src_ap(ci)).then_inc(
                    load_sem[ci % 2], 16
                )
                if ci >= 1:
                    sync.wait_ge(v_sem, ci)
                    sync.dma_start(
                        out=dst_ap(ci - 1), in_=wt[(ci - 1) % 2][:]
                    ).then_inc(store_sem[(ci - 1) % 2], 16)
            sync.wait_ge(v_sem, n_chunks)
            sync.dma_start(
                out=dst_ap(n_chunks - 1), in_=wt[(n_chunks - 1) % 2][:]
            ).then_inc(store_sem[(n_chunks - 1) % 2], 16)

        @block.gpsimd
        def _(gp):
            for ci in range(n_chunks):
                j = ci % 2
                xt4 = xt[j][:].rearrange("p (h q c) -> p h q c", h=H, c=C)
                wt4 = wt[j][:].rearrange("p (h q c) -> p h q c", h=H, c=C)

                # M2: out = p2 * 2*delta  (xt -> wt; wt dead after AND)
                d2_b = d2[j][:].unsqueeze(1).unsqueeze(3).broadcast_to([P, H, tt, C])
                gp.wait_ge(dve_sem, DVE_INCS * (ci + 1))        # AND(ci) done
                gp.tensor_tensor(out=wt4, in0=xt4, in1=d2_b, op=OP.mult).then_inc(
                    v_sem, 1
                )

        @block.vector
        def _(vector):
            for ci in range(n_chunks):
                j = ci % 2
                xt4 = xt[j][:].rearrange("p (h q c) -> p h q c", h=H, c=C)
                wt4 = wt[j][:].rearrange("p (h q c) -> p h q c", h=H, c=C)
                r13 = r1[:].rearrange("p (h q) -> p h q", h=H)

                if ci >= 1:
                    vector.wait_ge(dve_sem, DVE_INCS * ci)      # self-fence
                if ci >= 2:
                    vector.wait_ge(store_sem[j], 16 * (ci // 2))  # wt free
                vector.wait_ge(load_sem[j], 16 * (ci // 2 + 1))   # xt loaded

                # R1: per-(token,h) max over c; R2: delta = max over h
                vector.reduce_max(out=r13, in_=xt4, axis=X).then_inc(dve_sem, 1)
                vector.wait_ge(dve_sem, DVE_INCS * ci + 1)
                vector.reduce_max(
                    out=delta[:], in_=r13.transpose([0, 2, 1]), axis=X
                ).then_inc(dve_sem, 1)
                # per-token scalars
                vector.wait_ge(dve_sem, DVE_INCS * ci + 2)
                vector.tensor_scalar_mul(tf[:], delta[:], SQRT2).then_inc(dve_sem, 1)
                vector.wait_ge(dve_sem, DVE_INCS * ci + 3)
                vector.reciprocal(inv[:], tf[:]).then_inc(dve_sem, 1)
                vector.tensor_scalar_mul(d2[j][:], delta[:], 2.0).then_inc(dve_sem, 1)

                inv_b = inv[:].unsqueeze(1).unsqueeze(3).broadcast_to([P, H, tt, C])

                # M1: q = x * inv
                vector.wait_ge(dve_sem, DVE_INCS * ci + 5)
                vector.tensor_tensor(out=wt4, in0=xt4, in1=inv_b, op=OP.mult).then_inc(
                    dve_sem, 1
                )
                # AND: p2 = bits(q) & 0x7F800000  (wt -> xt, xt dead after M1)
                vector.wait_ge(dve_sem, DVE_INCS * ci + 6)
                vector.tensor_scalar(
                    out=xt[j][:].bitcast(i32),
                    in0=wt[j][:].bitcast(i32),
                    scalar1=EXP_MASK,
                    scalar2=None,
                    op0=OP.bitwise_and,
                ).then_inc(dve_sem, 1)

    _nc_cache["nc"] = nc
    return nc


def kernel(x: np.ndarray) -> np.ndarray:
    assert x.shape == (B, H, T, C) and x.dtype == np.float32
    nc = _build_nc()
    in_maps = [{"x": np.ascontiguousarray(x[i])} for i in range(N_CORES)]
    res = run_bass_kernel_spmd(nc, in_maps, list(range(N_CORES)))
    out = np.stack([res.results[i]["y"] for i in range(N_CORES)], axis=0)
    return out
